# revision 15
# baseline (speedup 1.0000x reference)
"""EquivariantDecoder GNN message-passing kernel for 8 Trainium2 NeuronCores.

Strategy (destination-sharded, collective-free):
  - Host sorts edges by dst node and shards by dst-node ranges: core k owns
    nodes [k*NK, (k+1)*NK) and ALL edges pointing into them. Each core fully
    computes its output rows; no cross-core reduction is needed.
  - Per-edge MLP  w = silu(m @ W1 + b1) @ W2 + b2  runs with W1 stationary on
    the tensor engine over host-pre-transposed m (mT: [H, Epad]).
  - The scatter-mean becomes, per 128-node window, an accumulation of
    one-hot matmuls in PSUM:  geomT[v, n] += rel'[e, v] * (w[e] * 1[dst==n]),
    where rel' = (x[src]-x[dst]) / max(cnt[dst], 1) is host-prepared and the
    scaled one-hot is built on the vector engine in one fused
    tensor_scalar((IOTA == dstrel) * w) op per 128-edge tile.
  - Node-side velocity gating alpha = silu(h @ vgW1 + b1) @ vgW2 + b2,
    vel_combo = sum_k alpha[:,k] * vel_all[:,k,:] is node-parallel.
  - Host unpacks/adds the two per-core outputs and concatenates.
"""

import hashlib
import os
import sys
import time

import numpy as np

sys.path.insert(0, "/opt/trn_rl_repo")

import ml_dtypes

# Namespace the neuron compile cache by this file's content: the cache keys
# on HLO module hashes, which do not see BIR-level kernel changes.
_SELF_HASH = hashlib.sha256(open(__file__, "rb").read()).hexdigest()[:16]
os.environ.setdefault(
    "NEURON_COMPILE_CACHE_URL", f"/tmp/neuron-cache-{_SELF_HASH}"
)

NC_CORES = 8
P = 128
H = 128
F32_EDGE = bool(int(os.environ.get("KERNEL_F32", "0")))

_COMPILED = {}  # (W, T_w, NKP) -> nc
LAST_EXEC_NS = None
LAST_RESULTS = None
TRACE = bool(int(os.environ.get("KERNEL_TRACE", "0")))


def _build_program(W, T_w, NKP):
    """Build + compile the SPMD Tile program for one core.

    W    : 128-node windows per core
    T_w  : tiles (128 edges) per window (uniform, host-padded)
    NKP  : node columns per core padded to a multiple of 512
    """
    from concourse import bacc, mybir, tile

    T = W * T_w          # edge tiles per core
    EPAD = T * P         # padded edge count per core

    f32 = mybir.dt.float32
    i32 = mybir.dt.int32
    ebf = f32 if F32_EDGE else mybir.dt.bfloat16

    nc = bacc.Bacc(
        "TRN2", target_bir_lowering=False, debug=False, num_devices=NC_CORES
    )

    # ---- DRAM I/O ----
    mT = nc.dram_tensor("mT", [P, EPAD], ebf, kind="ExternalInput").ap()
    relP = nc.dram_tensor("relP", [P, T * 3], ebf, kind="ExternalInput").ap()
    dstP = nc.dram_tensor("dstP", [P, T], ebf, kind="ExternalInput").ap()
    hT = nc.dram_tensor("hT", [P, NKP], f32, kind="ExternalInput").ap()
    velP = nc.dram_tensor("velP", [P, W * 15], f32, kind="ExternalInput").ap()
    ew_W1 = nc.dram_tensor("ew_W1", [P, H], ebf, kind="ExternalInput").ap()
    ew_b1 = nc.dram_tensor("ew_b1", [P, 1], f32, kind="ExternalInput").ap()
    ew_W2 = nc.dram_tensor("ew_W2", [P, 1], ebf, kind="ExternalInput").ap()
    ew_b2r = nc.dram_tensor("ew_b2r", [P, 1], f32, kind="ExternalInput").ap()
    vg_W1 = nc.dram_tensor("vg_W1", [P, H], f32, kind="ExternalInput").ap()
    vg_b1 = nc.dram_tensor("vg_b1", [P, 1], f32, kind="ExternalInput").ap()
    vg_W2 = nc.dram_tensor("vg_W2", [P, 5], f32, kind="ExternalInput").ap()
    vg_b2r = nc.dram_tensor("vg_b2r", [P, 5], f32, kind="ExternalInput").ap()
    geomT = nc.dram_tensor("geomT", [3, W * P], f32, kind="ExternalOutput").ap()
    vc = nc.dram_tensor("vc", [P, W * 3], f32, kind="ExternalOutput").ap()
    # Program-content nonce: makes the HLO fingerprint (and any HLO-keyed
    # executable cache) unique per kernel.py content.
    NONCE = (int(_SELF_HASH, 16) % 509) + 2
    nonce = nc.dram_tensor("nonce", [1, NONCE], f32, kind="ExternalInput").ap()

    MCH = 16       # mT chunk: 16 tiles = 2048 cols = 0.5 MiB bf16
    RCH = 128      # rel/dst chunk in tiles (must be a multiple of 4)

    Silu = mybir.ActivationFunctionType.Silu
    Copy = mybir.ActivationFunctionType.Copy
    add = mybir.AluOpType.add
    mult = mybir.AluOpType.mult
    is_equal = mybir.AluOpType.is_equal

    with tile.TileContext(nc) as tc:
        with (
            tc.tile_pool(name="const", bufs=1) as cpool,
            tc.tile_pool(name="mchunk", bufs=2) as mpool,
            tc.tile_pool(name="relchunk", bufs=2) as rpool,
            tc.tile_pool(name="silu", bufs=3) as spool,
            tc.tile_pool(name="wsb", bufs=3) as wpool,
            tc.tile_pool(name="oh", bufs=4) as ohpool,
            tc.tile_pool(name="acc", bufs=1) as accpool,
            tc.tile_pool(name="hchunk", bufs=2) as hpool,
            tc.tile_pool(name="nodesmall", bufs=3) as npool,
            tc.tile_pool(name="ps512", bufs=2, space="PSUM") as ps512,
            tc.tile_pool(name="pssmall", bufs=2, space="PSUM") as pssmall,
            tc.tile_pool(name="psgeom", bufs=2, space="PSUM") as psgeom,
        ):
            # ---- constants ----
            w1_sb = cpool.tile([P, H], ebf, tag="w1")
            nc.sync.dma_start(out=w1_sb[:], in_=ew_W1[:, :])
            b1_sb = cpool.tile([P, 1], f32, tag="b1")
            nc.sync.dma_start(out=b1_sb[:], in_=ew_b1[:, :])
            w2_sb = cpool.tile([P, 1], ebf, tag="w2")
            nc.sync.dma_start(out=w2_sb[:], in_=ew_W2[:, :])
            b2_sb = cpool.tile([P, 1], f32, tag="b2")
            nc.sync.dma_start(out=b2_sb[:], in_=ew_b2r[:, :])
            vw1_sb = cpool.tile([P, H], f32, tag="vw1")
            nc.sync.dma_start(out=vw1_sb[:], in_=vg_W1[:, :])
            vb1_sb = cpool.tile([P, 1], f32, tag="vb1")
            nc.sync.dma_start(out=vb1_sb[:], in_=vg_b1[:, :])
            vw2_sb = cpool.tile([P, 5], f32, tag="vw2")
            nc.sync.dma_start(out=vw2_sb[:], in_=vg_W2[:, :])
            vb2_sb = cpool.tile([P, 5], f32, tag="vb2")
            nc.sync.dma_start(out=vb2_sb[:], in_=vg_b2r[:, :])
            velP_sb = cpool.tile([P, W * 15], f32, tag="velp")
            nc.sync.dma_start(out=velP_sb[:], in_=velP[:, :])
            nonce_sb = cpool.tile([1, 512], f32, tag="nonce")
            nc.sync.dma_start(out=nonce_sb[:1, :NONCE], in_=nonce[:, :])

            # IOTA8: per-128 repeating iota over GT tiles, [128, GT*128]
            GT = 8  # tiles per edge group
            iota_i = cpool.tile([P, GT * P], i32, tag="iotai")
            nc.gpsimd.iota(
                iota_i[:], pattern=[[0, GT], [1, P]], base=0, channel_multiplier=0
            )
            iota_sb = cpool.tile([P, GT * P], ebf, tag="iotaf")
            nc.vector.tensor_copy(iota_sb[:], iota_i[:])

            geom_acc = accpool.tile([3, W * P], f32, tag="gacc")
            vc_acc = accpool.tile([P, W * 3], f32, tag="vacc")

            # ---- edge pipeline (groups of GT tiles = GT*128 edges) ----
            mch = None
            rch = None
            dch = None
            geom_ps = None
            for t0 in range(0, T, GT):
                gs = min(GT, T - t0)  # tiles in this group
                if t0 % MCH == 0:
                    mcols = min(MCH * P, EPAD - t0 * P)
                    mch = mpool.tile([P, MCH * P], ebf, tag="mch")
                    nc.sync.dma_start(
                        out=mch[:, :mcols], in_=mT[:, t0 * P : t0 * P + mcols]
                    )
                if t0 % RCH == 0:
                    rt = min(RCH, T - t0)
                    rch = rpool.tile([P, RCH * 3], ebf, tag="rch")
                    nc.sync.dma_start(
                        out=rch[:, : rt * 3], in_=relP[:, t0 * 3 : (t0 + rt) * 3]
                    )
                    dch = rpool.tile([P, RCH], ebf, tag="dch")
                    nc.sync.dma_start(out=dch[:, :rt], in_=dstP[:, t0 : t0 + rt])

                moff = (t0 % MCH) * P
                zT_ps = ps512.tile([P, GT * P], f32, tag="z512", space="PSUM")
                for c0 in range(0, gs * P, 512):  # one PSUM bank per matmul
                    cw = min(512, gs * P - c0)
                    nc.tensor.matmul(
                        out=zT_ps[:, c0 : c0 + cw],
                        lhsT=w1_sb[:],
                        rhs=mch[:, moff + c0 : moff + c0 + cw],
                        start=True,
                        stop=True,
                    )
                silu_sb = spool.tile([P, GT * P], ebf, tag="silu")
                nc.scalar.activation(
                    silu_sb[:, : gs * P], zT_ps[:, : gs * P], Silu, bias=b1_sb[:, :1]
                )

                w_ps = pssmall.tile([P, 8], f32, tag="wps", space="PSUM")
                for tt in range(gs):
                    nc.tensor.matmul(
                        out=w_ps[:, tt : tt + 1],
                        lhsT=silu_sb[:, tt * P : (tt + 1) * P],
                        rhs=w2_sb[:],
                        start=True,
                        stop=True,
                    )
                # relw[e, (t,c)] = (w_ps[e,t] + b2) * rel'[e, (t,c)]
                roff = (t0 % RCH) * 3
                relw_sb = wpool.tile([P, GT * 3], ebf, tag="relw")
                nc.vector.scalar_tensor_tensor(
                    out=relw_sb[:, : gs * 3].rearrange("p (t c) -> p t c", c=3),
                    in0=w_ps[:, :gs].unsqueeze(-1).broadcast_to([P, gs, 3]),
                    scalar=b2_sb[:, :1],
                    in1=rch[:, roff : roff + gs * 3].rearrange(
                        "p (t c) -> p t c", c=3
                    ),
                    op0=add,
                    op1=mult,
                )
                # eq[e, (t,n)] = (iota[n] == dstrel[e,t])  (one op per group)
                doff = t0 % RCH
                eq_sb = ohpool.tile([P, GT * P], ebf, tag="oh")
                nc.vector.tensor_tensor(
                    out=eq_sb[:, : gs * P].rearrange("p (t n) -> p t n", n=P),
                    in0=iota_sb[:, : gs * P].rearrange("p (t n) -> p t n", n=P),
                    in1=dch[:, doff : doff + gs]
                    .unsqueeze(-1)
                    .broadcast_to([P, gs, P]),
                    op=is_equal,
                )

                for tt in range(gs):
                    i = t0 + tt  # global tile
                    wwin = i // T_w
                    tin = i % T_w
                    if tin == 0:
                        geom_ps = psgeom.tile([3, P], f32, tag="gps", space="PSUM")
                    nc.tensor.matmul(
                        out=geom_ps[:],
                        lhsT=relw_sb[:, tt * 3 : (tt + 1) * 3],
                        rhs=eq_sb[:, tt * P : (tt + 1) * P],
                        start=(tin == 0),
                        stop=(tin == T_w - 1),
                    )
                    if tin == T_w - 1:
                        nc.scalar.activation(
                            geom_acc[:, wwin * P : (wwin + 1) * P],
                            geom_ps[:],
                            Copy,
                        )

            # ---- node pipeline ----
            NBN = NKP // 512
            HCH = 2048
            hch = None
            for b in range(NBN):
                c0 = b * 512
                if c0 % HCH == 0:
                    hcols = min(HCH, NKP - c0)
                    hch = hpool.tile([P, HCH], f32, tag="hch")
                    nc.sync.dma_start(
                        out=hch[:, :hcols], in_=hT[:, c0 : c0 + hcols]
                    )
                hoff = c0 % HCH
                z2_ps = ps512.tile([P, 512], f32, tag="z512", space="PSUM")
                nc.tensor.matmul(
                    out=z2_ps[:],
                    lhsT=vw1_sb[:],
                    rhs=hch[:, hoff : hoff + 512],
                    start=True,
                    stop=True,
                )
                silu2_sb = spool.tile([P, 512], f32, tag="silu2")
                nc.scalar.activation(silu2_sb[:], z2_ps[:], Silu, bias=vb1_sb[:, :1])
                for tt in range(4):
                    nt = b * 4 + tt  # node tile
                    if nt >= W:
                        break
                    a_ps = pssmall.tile([P, 8], f32, tag="wps", space="PSUM")
                    nc.tensor.matmul(
                        out=a_ps[:, :5],
                        lhsT=silu2_sb[:, tt * P : (tt + 1) * P],
                        rhs=vw2_sb[:],
                        start=True,
                        stop=True,
                    )
                    a_sb = npool.tile([P, 5], f32, tag="asb")
                    nc.vector.tensor_tensor(
                        out=a_sb[:], in0=a_ps[:, :5], in1=vb2_sb[:], op=add
                    )
                    velm = npool.tile([P, 15], f32, tag="velm")
                    nc.vector.tensor_tensor(
                        out=velm[:].rearrange("p (k v) -> p k v", v=3),
                        in0=velP_sb[:, nt * 15 : (nt + 1) * 15].rearrange(
                            "p (k v) -> p k v", v=3
                        ),
                        in1=a_sb[:].unsqueeze(-1).broadcast_to([P, 5, 3]),
                        op=mult,
                    )
                    nc.vector.tensor_reduce(
                        out=vc_acc[:, nt * 3 : (nt + 1) * 3],
                        in_=velm[:].rearrange("p (k v) -> p v k", v=3),
                        axis=mybir.AxisListType.X,
                        op=add,
                    )

            # ---- outputs ----
            nc.sync.dma_start(out=geomT[:, :], in_=geom_acc[:])
            nc.sync.dma_start(out=vc[:, :], in_=vc_acc[:])

    nc.compile()
    return nc


def _prep(h, m_ij, x, vel_all, edge_index, ew_W1, ew_b1, ew_W2, ew_b2,
          vg_W1, vg_b1, vg_W2, vg_b2):
    """Host-side sharding + layout packing. Returns (in_maps, meta)."""
    h = np.ascontiguousarray(np.asarray(h, dtype=np.float32))
    m_ij = np.ascontiguousarray(np.asarray(m_ij, dtype=np.float32))
    x = np.asarray(x, dtype=np.float32)
    vel_all = np.asarray(vel_all, dtype=np.float32)
    ei = np.asarray(edge_index)
    src = ei[0].astype(np.int64)
    dst = ei[1].astype(np.int64)

    N = h.shape[0]
    E = src.shape[0]

    W = int(np.ceil(N / (NC_CORES * P)))  # windows per core
    NK = W * P                            # nodes per core (padded)
    NPAD = NC_CORES * NK
    NKP = int(np.ceil(NK / 512)) * 512
    W_total = NC_CORES * W

    order = np.argsort(dst, kind="stable")
    dst_s = dst[order]
    cnt = np.bincount(dst, minlength=N).astype(np.float32)
    inv = 1.0 / np.maximum(cnt, 1.0)
    rel = (x[src] - x[dst]) * inv[dst][:, None]  # [E,3] with 1/cnt folded in

    wcnt = np.bincount(dst // P, minlength=W_total)
    T_w = max(int(np.ceil(wcnt.max() / P)), 1) if E > 0 else 1
    T = W * T_w
    EPAD = T * P

    win_starts = np.searchsorted(dst_s, np.arange(W_total) * P)
    offs = np.arange(T_w * P)
    slot_valid = offs[None, :] < wcnt[:, None]              # [W_total, T_w*P]
    slot_sorted = win_starts[:, None] + np.where(slot_valid, offs[None, :], 0)
    slot_sorted = np.minimum(slot_sorted, max(E - 1, 0))
    slot_eid = np.where(slot_valid, order[slot_sorted], -1)  # edge id or -1

    edt = np.float32 if F32_EDGE else ml_dtypes.bfloat16
    wt1 = np.ascontiguousarray(np.asarray(ew_W1, dtype=np.float32).astype(edt))
    wt2 = np.ascontiguousarray(
        np.asarray(ew_W2, dtype=np.float32).reshape(H, 1).astype(edt))
    vt1 = np.ascontiguousarray(vg_W1, dtype=np.float32)
    vt2 = np.ascontiguousarray(vg_W2, dtype=np.float32).reshape(H, 5)
    b1 = np.asarray(ew_b1, dtype=np.float32).reshape(H, 1)
    b2r = np.full((P, 1), np.float32(np.asarray(ew_b2).reshape(-1)[0]), np.float32)
    vb1 = np.asarray(vg_b1, dtype=np.float32).reshape(H, 1)
    vb2r = np.tile(np.asarray(vg_b2, dtype=np.float32).reshape(1, 5), (P, 1))

    h_pad = np.zeros((NPAD, H), np.float32)
    h_pad[:N] = h
    vel_pad = np.zeros((NPAD, 5, 3), np.float32)
    vel_pad[:N] = vel_all

    in_maps = []
    for k in range(NC_CORES):
        ids = slot_eid[k * W : (k + 1) * W].reshape(-1)  # [EPAD]
        valid = ids >= 0
        idc = np.where(valid, ids, 0)

        mg = m_ij[idc]
        mg[~valid] = 0.0
        mT = np.ascontiguousarray(mg.T.astype(edt))  # [H, EPAD]
        del mg

        rg = rel[idc]
        rg[~valid] = 0.0
        relP = np.ascontiguousarray(
            rg.reshape(T, P, 3).transpose(1, 0, 2).reshape(P, T * 3).astype(edt)
        )
        del rg

        base = (k * W + (np.arange(T) // T_w)) * P  # [T]
        dg = dst[idc].reshape(T, P) - base[:, None]
        dg[~valid.reshape(T, P)] = -1
        dstP = np.ascontiguousarray(dg.T.astype(np.float32).astype(edt))

        hT_k = np.zeros((H, NKP), np.float32)
        hT_k[:, :NK] = h_pad[k * NK : (k + 1) * NK].T
        velP_k = np.ascontiguousarray(
            vel_pad[k * NK : (k + 1) * NK]
            .reshape(W, P, 15)
            .transpose(1, 0, 2)
            .reshape(P, W * 15)
        )

        in_maps.append({
            "mT": mT, "relP": relP, "dstP": dstP, "hT": hT_k, "velP": velP_k,
            "ew_W1": wt1, "ew_b1": b1, "ew_W2": wt2, "ew_b2r": b2r,
            "vg_W1": vt1, "vg_b1": vb1, "vg_W2": vt2, "vg_b2r": vb2r,
            "nonce": np.zeros((1, (int(_SELF_HASH, 16) % 509) + 2), np.float32),
        })

    meta = dict(N=N, W=W, T_w=T_w, NK=NK, NKP=NKP)
    return in_maps, meta


def kernel(**inputs):
    global LAST_EXEC_NS, LAST_RESULTS
    from concourse.bass_utils import run_bass_kernel_spmd

    in_maps, meta = _prep(**inputs)
    key = (meta["W"], meta["T_w"], meta["NKP"])
    if key not in _COMPILED:
        _COMPILED[key] = _build_program(*key)
    nc = _COMPILED[key]

    t0 = time.time()
    res = run_bass_kernel_spmd(
        nc, in_maps, core_ids=list(range(NC_CORES)), trace=TRACE
    )
    LAST_EXEC_NS = res.exec_time_ns
    LAST_RESULTS = res
    _ = time.time() - t0

    N, W, NK = meta["N"], meta["W"], meta["NK"]
    parts = []
    for k in range(NC_CORES):
        r = res.results[k]
        g = r["geomT"].reshape(3, NK).T  # [NK,3]
        v = r["vc"].reshape(P, W, 3).transpose(1, 0, 2).reshape(NK, 3)
        parts.append(g + v)
    out = np.concatenate(parts, axis=0)[:N]
    return out.astype(np.float32)


# revision 16
# speedup vs baseline: 1.0832x; 1.0832x over previous
"""EquivariantDecoder GNN message-passing kernel for 8 Trainium2 NeuronCores.

Strategy (destination-sharded, collective-free):
  - Host sorts edges by dst node and shards by dst-node ranges: core k owns
    nodes [k*NK, (k+1)*NK) and ALL edges pointing into them. Each core fully
    computes its output rows; no cross-core reduction is needed.
  - Per-edge MLP  w = silu(m @ W1 + b1) @ W2 + b2  runs with W1 stationary on
    the tensor engine over host-pre-transposed m (mT: [H, Epad]).
  - The scatter-mean becomes, per 128-node window, an accumulation of
    one-hot matmuls in PSUM:  geomT[v, n] += rel'[e, v] * (w[e] * 1[dst==n]),
    where rel' = (x[src]-x[dst]) / max(cnt[dst], 1) is host-prepared and the
    scaled one-hot is built on the vector engine in one fused
    tensor_scalar((IOTA == dstrel) * w) op per 128-edge tile.
  - Node-side velocity gating alpha = silu(h @ vgW1 + b1) @ vgW2 + b2,
    vel_combo = sum_k alpha[:,k] * vel_all[:,k,:] is node-parallel.
  - Host unpacks/adds the two per-core outputs and concatenates.
"""

import hashlib
import os
import sys
import time

import numpy as np

sys.path.insert(0, "/opt/trn_rl_repo")

import ml_dtypes

# Namespace the neuron compile cache by this file's content: the cache keys
# on HLO module hashes, which do not see BIR-level kernel changes.
_SELF_HASH = hashlib.sha256(open(__file__, "rb").read()).hexdigest()[:16]
os.environ.setdefault(
    "NEURON_COMPILE_CACHE_URL", f"/tmp/neuron-cache-{_SELF_HASH}"
)

NC_CORES = 8
P = 128
H = 128
F32_EDGE = bool(int(os.environ.get("KERNEL_F32", "0")))

_COMPILED = {}  # (W, T_w, NKP) -> nc
LAST_EXEC_NS = None
LAST_RESULTS = None
TRACE = bool(int(os.environ.get("KERNEL_TRACE", "0")))


def _build_program(W, T_w, NKP):
    """Build + compile the SPMD Tile program for one core.

    W    : 128-node windows per core
    T_w  : tiles (128 edges) per window (uniform, host-padded)
    NKP  : node columns per core padded to a multiple of 512
    """
    from concourse import bacc, mybir, tile

    T = W * T_w          # edge tiles per core
    EPAD = T * P         # padded edge count per core

    f32 = mybir.dt.float32
    i32 = mybir.dt.int32
    ebf = f32 if F32_EDGE else mybir.dt.bfloat16

    nc = bacc.Bacc(
        "TRN2", target_bir_lowering=False, debug=False, num_devices=NC_CORES
    )

    # ---- DRAM I/O ----
    mT = nc.dram_tensor("mT", [P, EPAD], ebf, kind="ExternalInput").ap()
    relP = nc.dram_tensor("relP", [P, T * 3], ebf, kind="ExternalInput").ap()
    dstP = nc.dram_tensor("dstP", [P, T], ebf, kind="ExternalInput").ap()
    hT = nc.dram_tensor("hT", [P, NKP], f32, kind="ExternalInput").ap()
    velP = nc.dram_tensor("velP", [P, W * 15], f32, kind="ExternalInput").ap()
    ew_W1 = nc.dram_tensor("ew_W1", [P, H], ebf, kind="ExternalInput").ap()
    ew_b1 = nc.dram_tensor("ew_b1", [P, 1], f32, kind="ExternalInput").ap()
    ew_W2 = nc.dram_tensor("ew_W2", [P, 1], ebf, kind="ExternalInput").ap()
    ew_b2r = nc.dram_tensor("ew_b2r", [P, 1], f32, kind="ExternalInput").ap()
    vg_W1 = nc.dram_tensor("vg_W1", [P, H], f32, kind="ExternalInput").ap()
    vg_b1 = nc.dram_tensor("vg_b1", [P, 1], f32, kind="ExternalInput").ap()
    vg_W2 = nc.dram_tensor("vg_W2", [P, 5], f32, kind="ExternalInput").ap()
    vg_b2r = nc.dram_tensor("vg_b2r", [P, 5], f32, kind="ExternalInput").ap()
    geomT = nc.dram_tensor("geomT", [3, W * P], f32, kind="ExternalOutput").ap()
    vc = nc.dram_tensor("vc", [P, W * 3], f32, kind="ExternalOutput").ap()
    # Program-content nonce: makes the HLO fingerprint (and any HLO-keyed
    # executable cache) unique per kernel.py content.
    NONCE = (int(_SELF_HASH, 16) % 509) + 2
    nonce = nc.dram_tensor("nonce", [1, NONCE], f32, kind="ExternalInput").ap()

    MCH = 64       # mT chunk: 64 tiles = 8192 cols = 2 MiB bf16
    RCH = 128      # rel/dst chunk in tiles (must be a multiple of 4)

    Silu = mybir.ActivationFunctionType.Silu
    Copy = mybir.ActivationFunctionType.Copy
    add = mybir.AluOpType.add
    mult = mybir.AluOpType.mult
    is_equal = mybir.AluOpType.is_equal

    with tile.TileContext(nc) as tc:
        with (
            tc.tile_pool(name="const", bufs=1) as cpool,
            tc.tile_pool(name="mchunk", bufs=2) as mpool,
            tc.tile_pool(name="relchunk", bufs=2) as rpool,
            tc.tile_pool(name="silu", bufs=3) as spool,
            tc.tile_pool(name="wsb", bufs=3) as wpool,
            tc.tile_pool(name="oh", bufs=4) as ohpool,
            tc.tile_pool(name="acc", bufs=1) as accpool,
            tc.tile_pool(name="hchunk", bufs=2) as hpool,
            tc.tile_pool(name="nodesmall", bufs=3) as npool,
            tc.tile_pool(name="ps512", bufs=2, space="PSUM") as ps512,
            tc.tile_pool(name="pssmall", bufs=2, space="PSUM") as pssmall,
            tc.tile_pool(name="psgeom", bufs=2, space="PSUM") as psgeom,
        ):
            # ---- constants ----
            w1_sb = cpool.tile([P, H], ebf, tag="w1")
            nc.sync.dma_start(out=w1_sb[:], in_=ew_W1[:, :])
            b1_sb = cpool.tile([P, 1], f32, tag="b1")
            nc.sync.dma_start(out=b1_sb[:], in_=ew_b1[:, :])
            w2_sb = cpool.tile([P, 1], ebf, tag="w2")
            nc.sync.dma_start(out=w2_sb[:], in_=ew_W2[:, :])
            b2_sb = cpool.tile([P, 1], f32, tag="b2")
            nc.sync.dma_start(out=b2_sb[:], in_=ew_b2r[:, :])
            vw1_sb = cpool.tile([P, H], f32, tag="vw1")
            nc.sync.dma_start(out=vw1_sb[:], in_=vg_W1[:, :])
            vb1_sb = cpool.tile([P, 1], f32, tag="vb1")
            nc.sync.dma_start(out=vb1_sb[:], in_=vg_b1[:, :])
            vw2_sb = cpool.tile([P, 5], f32, tag="vw2")
            nc.sync.dma_start(out=vw2_sb[:], in_=vg_W2[:, :])
            vb2_sb = cpool.tile([P, 5], f32, tag="vb2")
            nc.sync.dma_start(out=vb2_sb[:], in_=vg_b2r[:, :])
            velP_sb = cpool.tile([P, W * 15], f32, tag="velp")
            nc.sync.dma_start(out=velP_sb[:], in_=velP[:, :])
            nonce_sb = cpool.tile([1, 512], f32, tag="nonce")
            nc.sync.dma_start(out=nonce_sb[:1, :NONCE], in_=nonce[:, :])

            # IOTA8: per-128 repeating iota over GT tiles, [128, GT*128]
            GT = 8  # tiles per edge group
            iota_i = cpool.tile([P, GT * P], i32, tag="iotai")
            nc.gpsimd.iota(
                iota_i[:], pattern=[[0, GT], [1, P]], base=0, channel_multiplier=0
            )
            iota_sb = cpool.tile([P, GT * P], ebf, tag="iotaf")
            nc.vector.tensor_copy(iota_sb[:], iota_i[:])

            geom_acc = accpool.tile([3, W * P], f32, tag="gacc")
            vc_acc = accpool.tile([P, W * 3], f32, tag="vacc")

            # ---- edge pipeline (groups of GT tiles = GT*128 edges) ----
            mch = None
            rch = None
            dch = None
            geom_ps = None
            for t0 in range(0, T, GT):
                gs = min(GT, T - t0)  # tiles in this group
                if t0 % MCH == 0:
                    mcols = min(MCH * P, EPAD - t0 * P)
                    mch = mpool.tile([P, MCH * P], ebf, tag="mch")
                    nc.sync.dma_start(
                        out=mch[:, :mcols], in_=mT[:, t0 * P : t0 * P + mcols]
                    )
                if t0 % RCH == 0:
                    rt = min(RCH, T - t0)
                    rch = rpool.tile([P, RCH * 3], ebf, tag="rch")
                    nc.sync.dma_start(
                        out=rch[:, : rt * 3], in_=relP[:, t0 * 3 : (t0 + rt) * 3]
                    )
                    dch = rpool.tile([P, RCH], ebf, tag="dch")
                    nc.sync.dma_start(out=dch[:, :rt], in_=dstP[:, t0 : t0 + rt])

                moff = (t0 % MCH) * P
                zT_ps = ps512.tile([P, GT * P], f32, tag="z512", space="PSUM")
                for c0 in range(0, gs * P, 512):  # one PSUM bank per matmul
                    cw = min(512, gs * P - c0)
                    nc.tensor.matmul(
                        out=zT_ps[:, c0 : c0 + cw],
                        lhsT=w1_sb[:],
                        rhs=mch[:, moff + c0 : moff + c0 + cw],
                        start=True,
                        stop=True,
                    )
                silu_sb = spool.tile([P, GT * P], ebf, tag="silu")
                nc.scalar.activation(
                    silu_sb[:, : gs * P], zT_ps[:, : gs * P], Silu, bias=b1_sb[:, :1]
                )

                w_ps = pssmall.tile([P, 8], f32, tag="wps", space="PSUM")
                for tt in range(gs):
                    nc.tensor.matmul(
                        out=w_ps[:, tt : tt + 1],
                        lhsT=silu_sb[:, tt * P : (tt + 1) * P],
                        rhs=w2_sb[:],
                        start=True,
                        stop=True,
                    )
                # relw[e, (t,c)] = (w_ps[e,t] + b2) * rel'[e, (t,c)]
                roff = (t0 % RCH) * 3
                relw_sb = wpool.tile([P, GT * 3], ebf, tag="relw")
                nc.vector.scalar_tensor_tensor(
                    out=relw_sb[:, : gs * 3].rearrange("p (t c) -> p t c", c=3),
                    in0=w_ps[:, :gs].unsqueeze(-1).broadcast_to([P, gs, 3]),
                    scalar=b2_sb[:, :1],
                    in1=rch[:, roff : roff + gs * 3].rearrange(
                        "p (t c) -> p t c", c=3
                    ),
                    op0=add,
                    op1=mult,
                )
                # eq[e, (t,n)] = (iota[n] == dstrel[e,t])  (one op per group)
                doff = t0 % RCH
                eq_sb = ohpool.tile([P, GT * P], ebf, tag="oh")
                nc.vector.tensor_tensor(
                    out=eq_sb[:, : gs * P].rearrange("p (t n) -> p t n", n=P),
                    in0=iota_sb[:, : gs * P].rearrange("p (t n) -> p t n", n=P),
                    in1=dch[:, doff : doff + gs]
                    .unsqueeze(-1)
                    .broadcast_to([P, gs, P]),
                    op=is_equal,
                )

                for tt in range(gs):
                    i = t0 + tt  # global tile
                    wwin = i // T_w
                    tin = i % T_w
                    if tin == 0:
                        geom_ps = psgeom.tile([3, P], f32, tag="gps", space="PSUM")
                    nc.tensor.matmul(
                        out=geom_ps[:],
                        lhsT=relw_sb[:, tt * 3 : (tt + 1) * 3],
                        rhs=eq_sb[:, tt * P : (tt + 1) * P],
                        start=(tin == 0),
                        stop=(tin == T_w - 1),
                    )
                    if tin == T_w - 1:
                        nc.scalar.activation(
                            geom_acc[:, wwin * P : (wwin + 1) * P],
                            geom_ps[:],
                            Copy,
                        )

            # ---- node pipeline ----
            NBN = NKP // 512
            HCH = 2048
            hch = None
            for b in range(NBN):
                c0 = b * 512
                if c0 % HCH == 0:
                    hcols = min(HCH, NKP - c0)
                    hch = hpool.tile([P, HCH], f32, tag="hch")
                    nc.sync.dma_start(
                        out=hch[:, :hcols], in_=hT[:, c0 : c0 + hcols]
                    )
                hoff = c0 % HCH
                z2_ps = ps512.tile([P, 512], f32, tag="z512", space="PSUM")
                nc.tensor.matmul(
                    out=z2_ps[:],
                    lhsT=vw1_sb[:],
                    rhs=hch[:, hoff : hoff + 512],
                    start=True,
                    stop=True,
                )
                silu2_sb = spool.tile([P, 512], f32, tag="silu2")
                nc.scalar.activation(silu2_sb[:], z2_ps[:], Silu, bias=vb1_sb[:, :1])
                for tt in range(4):
                    nt = b * 4 + tt  # node tile
                    if nt >= W:
                        break
                    a_ps = pssmall.tile([P, 8], f32, tag="wps", space="PSUM")
                    nc.tensor.matmul(
                        out=a_ps[:, :5],
                        lhsT=silu2_sb[:, tt * P : (tt + 1) * P],
                        rhs=vw2_sb[:],
                        start=True,
                        stop=True,
                    )
                    a_sb = npool.tile([P, 5], f32, tag="asb")
                    nc.vector.tensor_tensor(
                        out=a_sb[:], in0=a_ps[:, :5], in1=vb2_sb[:], op=add
                    )
                    velm = npool.tile([P, 15], f32, tag="velm")
                    nc.vector.tensor_tensor(
                        out=velm[:].rearrange("p (k v) -> p k v", v=3),
                        in0=velP_sb[:, nt * 15 : (nt + 1) * 15].rearrange(
                            "p (k v) -> p k v", v=3
                        ),
                        in1=a_sb[:].unsqueeze(-1).broadcast_to([P, 5, 3]),
                        op=mult,
                    )
                    nc.vector.tensor_reduce(
                        out=vc_acc[:, nt * 3 : (nt + 1) * 3],
                        in_=velm[:].rearrange("p (k v) -> p v k", v=3),
                        axis=mybir.AxisListType.X,
                        op=add,
                    )

            # ---- outputs ----
            nc.sync.dma_start(out=geomT[:, :], in_=geom_acc[:])
            nc.sync.dma_start(out=vc[:, :], in_=vc_acc[:])

    nc.compile()
    return nc


def _prep(h, m_ij, x, vel_all, edge_index, ew_W1, ew_b1, ew_W2, ew_b2,
          vg_W1, vg_b1, vg_W2, vg_b2):
    """Host-side sharding + layout packing. Returns (in_maps, meta)."""
    h = np.ascontiguousarray(np.asarray(h, dtype=np.float32))
    m_ij = np.ascontiguousarray(np.asarray(m_ij, dtype=np.float32))
    x = np.asarray(x, dtype=np.float32)
    vel_all = np.asarray(vel_all, dtype=np.float32)
    ei = np.asarray(edge_index)
    src = ei[0].astype(np.int64)
    dst = ei[1].astype(np.int64)

    N = h.shape[0]
    E = src.shape[0]

    W = int(np.ceil(N / (NC_CORES * P)))  # windows per core
    NK = W * P                            # nodes per core (padded)
    NPAD = NC_CORES * NK
    NKP = int(np.ceil(NK / 512)) * 512
    W_total = NC_CORES * W

    order = np.argsort(dst, kind="stable")
    dst_s = dst[order]
    cnt = np.bincount(dst, minlength=N).astype(np.float32)
    inv = 1.0 / np.maximum(cnt, 1.0)
    rel = (x[src] - x[dst]) * inv[dst][:, None]  # [E,3] with 1/cnt folded in

    wcnt = np.bincount(dst // P, minlength=W_total)
    T_w = max(int(np.ceil(wcnt.max() / P)), 1) if E > 0 else 1
    T = W * T_w
    EPAD = T * P

    win_starts = np.searchsorted(dst_s, np.arange(W_total) * P)
    offs = np.arange(T_w * P)
    slot_valid = offs[None, :] < wcnt[:, None]              # [W_total, T_w*P]
    slot_sorted = win_starts[:, None] + np.where(slot_valid, offs[None, :], 0)
    slot_sorted = np.minimum(slot_sorted, max(E - 1, 0))
    slot_eid = np.where(slot_valid, order[slot_sorted], -1)  # edge id or -1

    edt = np.float32 if F32_EDGE else ml_dtypes.bfloat16
    wt1 = np.ascontiguousarray(np.asarray(ew_W1, dtype=np.float32).astype(edt))
    wt2 = np.ascontiguousarray(
        np.asarray(ew_W2, dtype=np.float32).reshape(H, 1).astype(edt))
    vt1 = np.ascontiguousarray(vg_W1, dtype=np.float32)
    vt2 = np.ascontiguousarray(vg_W2, dtype=np.float32).reshape(H, 5)
    b1 = np.asarray(ew_b1, dtype=np.float32).reshape(H, 1)
    b2r = np.full((P, 1), np.float32(np.asarray(ew_b2).reshape(-1)[0]), np.float32)
    vb1 = np.asarray(vg_b1, dtype=np.float32).reshape(H, 1)
    vb2r = np.tile(np.asarray(vg_b2, dtype=np.float32).reshape(1, 5), (P, 1))

    h_pad = np.zeros((NPAD, H), np.float32)
    h_pad[:N] = h
    vel_pad = np.zeros((NPAD, 5, 3), np.float32)
    vel_pad[:N] = vel_all

    in_maps = []
    for k in range(NC_CORES):
        ids = slot_eid[k * W : (k + 1) * W].reshape(-1)  # [EPAD]
        valid = ids >= 0
        idc = np.where(valid, ids, 0)

        mg = m_ij[idc]
        mg[~valid] = 0.0
        mT = np.ascontiguousarray(mg.T.astype(edt))  # [H, EPAD]
        del mg

        rg = rel[idc]
        rg[~valid] = 0.0
        relP = np.ascontiguousarray(
            rg.reshape(T, P, 3).transpose(1, 0, 2).reshape(P, T * 3).astype(edt)
        )
        del rg

        base = (k * W + (np.arange(T) // T_w)) * P  # [T]
        dg = dst[idc].reshape(T, P) - base[:, None]
        dg[~valid.reshape(T, P)] = -1
        dstP = np.ascontiguousarray(dg.T.astype(np.float32).astype(edt))

        hT_k = np.zeros((H, NKP), np.float32)
        hT_k[:, :NK] = h_pad[k * NK : (k + 1) * NK].T
        velP_k = np.ascontiguousarray(
            vel_pad[k * NK : (k + 1) * NK]
            .reshape(W, P, 15)
            .transpose(1, 0, 2)
            .reshape(P, W * 15)
        )

        in_maps.append({
            "mT": mT, "relP": relP, "dstP": dstP, "hT": hT_k, "velP": velP_k,
            "ew_W1": wt1, "ew_b1": b1, "ew_W2": wt2, "ew_b2r": b2r,
            "vg_W1": vt1, "vg_b1": vb1, "vg_W2": vt2, "vg_b2r": vb2r,
            "nonce": np.zeros((1, (int(_SELF_HASH, 16) % 509) + 2), np.float32),
        })

    meta = dict(N=N, W=W, T_w=T_w, NK=NK, NKP=NKP)
    return in_maps, meta


def kernel(**inputs):
    global LAST_EXEC_NS, LAST_RESULTS
    from concourse.bass_utils import run_bass_kernel_spmd

    in_maps, meta = _prep(**inputs)
    key = (meta["W"], meta["T_w"], meta["NKP"])
    if key not in _COMPILED:
        _COMPILED[key] = _build_program(*key)
    nc = _COMPILED[key]

    t0 = time.time()
    res = run_bass_kernel_spmd(
        nc, in_maps, core_ids=list(range(NC_CORES)), trace=TRACE
    )
    LAST_EXEC_NS = res.exec_time_ns
    LAST_RESULTS = res
    _ = time.time() - t0

    N, W, NK = meta["N"], meta["W"], meta["NK"]
    parts = []
    for k in range(NC_CORES):
        r = res.results[k]
        g = r["geomT"].reshape(3, NK).T  # [NK,3]
        v = r["vc"].reshape(P, W, 3).transpose(1, 0, 2).reshape(NK, 3)
        parts.append(g + v)
    out = np.concatenate(parts, axis=0)[:N]
    return out.astype(np.float32)


# revision 18
# speedup vs baseline: 1.1980x; 1.1059x over previous
"""EquivariantDecoder GNN message-passing kernel for 8 Trainium2 NeuronCores.

Strategy (destination-sharded, collective-free):
  - Host sorts edges by dst node and shards by dst-node ranges: core k owns
    nodes [k*NK, (k+1)*NK) and ALL edges pointing into them. Each core fully
    computes its output rows; no cross-core reduction is needed.
  - Per-edge MLP  w = silu(m @ W1 + b1) @ W2 + b2  runs with W1 stationary on
    the tensor engine over host-pre-transposed m (mT: [H, Epad]).
  - The scatter-mean becomes, per 128-node window, an accumulation of
    one-hot matmuls in PSUM:  geomT[v, n] += rel'[e, v] * (w[e] * 1[dst==n]),
    where rel' = (x[src]-x[dst]) / max(cnt[dst], 1) is host-prepared and the
    scaled one-hot is built on the vector engine in one fused
    tensor_scalar((IOTA == dstrel) * w) op per 128-edge tile.
  - Node-side velocity gating alpha = silu(h @ vgW1 + b1) @ vgW2 + b2,
    vel_combo = sum_k alpha[:,k] * vel_all[:,k,:] is node-parallel.
  - Host unpacks/adds the two per-core outputs and concatenates.
"""

import hashlib
import os
import sys
import time

import numpy as np

sys.path.insert(0, "/opt/trn_rl_repo")

import ml_dtypes

# Namespace the neuron compile cache by this file's content: the cache keys
# on HLO module hashes, which do not see BIR-level kernel changes.
_SELF_HASH = hashlib.sha256(open(__file__, "rb").read()).hexdigest()[:16]
os.environ.setdefault(
    "NEURON_COMPILE_CACHE_URL", f"/tmp/neuron-cache-{_SELF_HASH}"
)

NC_CORES = 8
P = 128
H = 128
F32_EDGE = bool(int(os.environ.get("KERNEL_F32", "0")))

_COMPILED = {}  # (W, T_w, NKP) -> nc
LAST_EXEC_NS = None
LAST_RESULTS = None
TRACE = bool(int(os.environ.get("KERNEL_TRACE", "0")))


def _build_program(W, T_w, NKP):
    """Build + compile the SPMD Tile program for one core.

    W    : 128-node windows per core
    T_w  : tiles (128 edges) per window (uniform, host-padded)
    NKP  : node columns per core padded to a multiple of 512
    """
    from concourse import bacc, mybir, tile

    T = W * T_w          # edge tiles per core
    EPAD = T * P         # padded edge count per core

    f32 = mybir.dt.float32
    i32 = mybir.dt.int32
    ebf = f32 if F32_EDGE else mybir.dt.bfloat16

    nc = bacc.Bacc(
        "TRN2", target_bir_lowering=False, debug=False, num_devices=NC_CORES
    )

    # ---- DRAM I/O ----
    mT = nc.dram_tensor("mT", [P, EPAD], ebf, kind="ExternalInput").ap()
    relP = nc.dram_tensor("relP", [P, T * 3], ebf, kind="ExternalInput").ap()
    dstP = nc.dram_tensor("dstP", [P, T], ebf, kind="ExternalInput").ap()
    hT = nc.dram_tensor("hT", [P, NKP], ebf, kind="ExternalInput").ap()
    velP = nc.dram_tensor("velP", [P, W * 15], f32, kind="ExternalInput").ap()
    ew_W1 = nc.dram_tensor("ew_W1", [P, H], ebf, kind="ExternalInput").ap()
    ew_b1 = nc.dram_tensor("ew_b1", [P, 1], f32, kind="ExternalInput").ap()
    ew_W2 = nc.dram_tensor("ew_W2", [P, 1], ebf, kind="ExternalInput").ap()
    ew_b2r = nc.dram_tensor("ew_b2r", [P, 1], f32, kind="ExternalInput").ap()
    vg_W1 = nc.dram_tensor("vg_W1", [P, H], ebf, kind="ExternalInput").ap()
    vg_b1 = nc.dram_tensor("vg_b1", [P, 1], f32, kind="ExternalInput").ap()
    vg_W2 = nc.dram_tensor("vg_W2", [P, 5], ebf, kind="ExternalInput").ap()
    vg_b2r = nc.dram_tensor("vg_b2r", [P, 5], f32, kind="ExternalInput").ap()
    geomT = nc.dram_tensor("geomT", [3, W * P], f32, kind="ExternalOutput").ap()
    vc = nc.dram_tensor("vc", [P, W * 3], f32, kind="ExternalOutput").ap()
    # Program-content nonce: makes the HLO fingerprint (and any HLO-keyed
    # executable cache) unique per kernel.py content.
    NONCE = (int(_SELF_HASH, 16) % 509) + 2
    nonce = nc.dram_tensor("nonce", [1, NONCE], f32, kind="ExternalInput").ap()

    MCH = 64       # mT chunk: 64 tiles = 8192 cols = 2 MiB bf16
    RCH = 128      # rel/dst chunk in tiles (must be a multiple of 4)

    Silu = mybir.ActivationFunctionType.Silu
    Copy = mybir.ActivationFunctionType.Copy
    add = mybir.AluOpType.add
    mult = mybir.AluOpType.mult
    is_equal = mybir.AluOpType.is_equal

    with tile.TileContext(nc) as tc:
        with (
            tc.tile_pool(name="const", bufs=1) as cpool,
            tc.tile_pool(name="mchunk", bufs=2) as mpool,
            tc.tile_pool(name="relchunk", bufs=2) as rpool,
            tc.tile_pool(name="silu", bufs=3) as spool,
            tc.tile_pool(name="wsb", bufs=3) as wpool,
            tc.tile_pool(name="oh", bufs=4) as ohpool,
            tc.tile_pool(name="acc", bufs=1) as accpool,
            tc.tile_pool(name="hchunk", bufs=2) as hpool,
            tc.tile_pool(name="nodesmall", bufs=3) as npool,
            tc.tile_pool(name="ps512", bufs=2, space="PSUM") as ps512,
            tc.tile_pool(name="pssmall", bufs=2, space="PSUM") as pssmall,
            tc.tile_pool(name="psgeom", bufs=2, space="PSUM") as psgeom,
        ):
            # ---- constants ----
            w1_sb = cpool.tile([P, H], ebf, tag="w1")
            nc.sync.dma_start(out=w1_sb[:], in_=ew_W1[:, :])
            b1_sb = cpool.tile([P, 1], f32, tag="b1")
            nc.sync.dma_start(out=b1_sb[:], in_=ew_b1[:, :])
            w2_sb = cpool.tile([P, 1], ebf, tag="w2")
            nc.sync.dma_start(out=w2_sb[:], in_=ew_W2[:, :])
            b2_sb = cpool.tile([P, 1], f32, tag="b2")
            nc.sync.dma_start(out=b2_sb[:], in_=ew_b2r[:, :])
            vw1_sb = cpool.tile([P, H], ebf, tag="vw1")
            nc.sync.dma_start(out=vw1_sb[:], in_=vg_W1[:, :])
            vb1_sb = cpool.tile([P, 1], f32, tag="vb1")
            nc.sync.dma_start(out=vb1_sb[:], in_=vg_b1[:, :])
            vw2_sb = cpool.tile([P, 5], ebf, tag="vw2")
            nc.sync.dma_start(out=vw2_sb[:], in_=vg_W2[:, :])
            vb2_sb = cpool.tile([P, 5], f32, tag="vb2")
            nc.sync.dma_start(out=vb2_sb[:], in_=vg_b2r[:, :])
            velP_sb = cpool.tile([P, W * 15], f32, tag="velp")
            nc.sync.dma_start(out=velP_sb[:], in_=velP[:, :])
            nonce_sb = cpool.tile([1, 512], f32, tag="nonce")
            nc.sync.dma_start(out=nonce_sb[:1, :NONCE], in_=nonce[:, :])

            # IOTA8: per-128 repeating iota over GT tiles, [128, GT*128]
            GT = 8  # tiles per edge group
            iota_i = cpool.tile([P, GT * P], i32, tag="iotai")
            nc.gpsimd.iota(
                iota_i[:], pattern=[[0, GT], [1, P]], base=0, channel_multiplier=0
            )
            iota_sb = cpool.tile([P, GT * P], ebf, tag="iotaf")
            nc.vector.tensor_copy(iota_sb[:], iota_i[:])

            geom_acc = accpool.tile([3, W * P], f32, tag="gacc")
            vc_acc = accpool.tile([P, W * 3], f32, tag="vacc")

            # ---- edge pipeline (groups of GT tiles = GT*128 edges) ----
            mch = None
            rch = None
            dch = None
            geom_ps = None
            for t0 in range(0, T, GT):
                gs = min(GT, T - t0)  # tiles in this group
                if t0 % MCH == 0:
                    mcols = min(MCH * P, EPAD - t0 * P)
                    mch = mpool.tile([P, MCH * P], ebf, tag="mch")
                    nc.sync.dma_start(
                        out=mch[:, :mcols], in_=mT[:, t0 * P : t0 * P + mcols]
                    )
                if t0 % RCH == 0:
                    rt = min(RCH, T - t0)
                    rch = rpool.tile([P, RCH * 3], ebf, tag="rch")
                    nc.sync.dma_start(
                        out=rch[:, : rt * 3], in_=relP[:, t0 * 3 : (t0 + rt) * 3]
                    )
                    dch = rpool.tile([P, RCH], ebf, tag="dch")
                    nc.sync.dma_start(out=dch[:, :rt], in_=dstP[:, t0 : t0 + rt])

                moff = (t0 % MCH) * P
                zT_ps = ps512.tile([P, GT * P], f32, tag="z512", space="PSUM")
                for c0 in range(0, gs * P, 512):  # one PSUM bank per matmul
                    cw = min(512, gs * P - c0)
                    nc.tensor.matmul(
                        out=zT_ps[:, c0 : c0 + cw],
                        lhsT=w1_sb[:],
                        rhs=mch[:, moff + c0 : moff + c0 + cw],
                        start=True,
                        stop=True,
                    )
                silu_sb = spool.tile([P, GT * P], ebf, tag="silu")
                nc.scalar.activation(
                    silu_sb[:, : gs * P], zT_ps[:, : gs * P], Silu, bias=b1_sb[:, :1]
                )

                w_ps = pssmall.tile([P, 8], f32, tag="wps", space="PSUM")
                for tt in range(gs):
                    nc.tensor.matmul(
                        out=w_ps[:, tt : tt + 1],
                        lhsT=silu_sb[:, tt * P : (tt + 1) * P],
                        rhs=w2_sb[:],
                        start=True,
                        stop=True,
                    )
                # w = W2-dot + b2, cast to bf16 (one op per group)
                w_sb = wpool.tile([P, GT], ebf, tag="wsb")
                nc.vector.tensor_scalar(
                    w_sb[:, :gs], w_ps[:, :gs], b2_sb[:, :1], None, add
                )
                # relw[e, (t,c)] = rel'[e, (t,c)] * w[e, t]  (one op per group)
                roff = (t0 % RCH) * 3
                relw_sb = wpool.tile([P, GT * 3], ebf, tag="relw")
                nc.vector.scalar_tensor_tensor(
                    out=relw_sb[:, : gs * 3].rearrange("p (t c) -> p t c", c=3),
                    in0=rch[:, roff : roff + gs * 3].rearrange(
                        "p (t c) -> p t c", c=3
                    ),
                    scalar=1.0,
                    in1=w_sb[:, :gs].unsqueeze(-1).broadcast_to([P, gs, 3]),
                    op0=mult,
                    op1=mult,
                )
                # eq[e, (t,n)] = (iota[n] == dstrel[e,t])  (one op per group)
                doff = t0 % RCH
                eq_sb = ohpool.tile([P, GT * P], ebf, tag="oh")
                nc.vector.tensor_tensor(
                    out=eq_sb[:, : gs * P].rearrange("p (t n) -> p t n", n=P),
                    in0=iota_sb[:, : gs * P].rearrange("p (t n) -> p t n", n=P),
                    in1=dch[:, doff : doff + gs]
                    .unsqueeze(-1)
                    .broadcast_to([P, gs, P]),
                    op=is_equal,
                )

                for tt in range(gs):
                    i = t0 + tt  # global tile
                    wwin = i // T_w
                    tin = i % T_w
                    if tin == 0:
                        geom_ps = psgeom.tile([3, P], f32, tag="gps", space="PSUM")
                    nc.tensor.matmul(
                        out=geom_ps[:],
                        lhsT=relw_sb[:, tt * 3 : (tt + 1) * 3],
                        rhs=eq_sb[:, tt * P : (tt + 1) * P],
                        start=(tin == 0),
                        stop=(tin == T_w - 1),
                    )
                    if tin == T_w - 1:
                        nc.scalar.activation(
                            geom_acc[:, wwin * P : (wwin + 1) * P],
                            geom_ps[:],
                            Copy,
                        )

            # ---- node pipeline ----
            NBN = NKP // 512
            HCH = 2048
            hch = None
            for b in range(NBN):
                c0 = b * 512
                if c0 % HCH == 0:
                    hcols = min(HCH, NKP - c0)
                    hch = hpool.tile([P, HCH], ebf, tag="hch")
                    nc.sync.dma_start(
                        out=hch[:, :hcols], in_=hT[:, c0 : c0 + hcols]
                    )
                hoff = c0 % HCH
                z2_ps = ps512.tile([P, 512], f32, tag="z512", space="PSUM")
                nc.tensor.matmul(
                    out=z2_ps[:],
                    lhsT=vw1_sb[:],
                    rhs=hch[:, hoff : hoff + 512],
                    start=True,
                    stop=True,
                )
                silu2_sb = spool.tile([P, 512], ebf, tag="silu2")
                nc.scalar.activation(silu2_sb[:], z2_ps[:], Silu, bias=vb1_sb[:, :1])
                for tt in range(4):
                    nt = b * 4 + tt  # node tile
                    if nt >= W:
                        break
                    a_ps = pssmall.tile([P, 8], f32, tag="wps", space="PSUM")
                    nc.tensor.matmul(
                        out=a_ps[:, :5],
                        lhsT=silu2_sb[:, tt * P : (tt + 1) * P],
                        rhs=vw2_sb[:],
                        start=True,
                        stop=True,
                    )
                    a_sb = npool.tile([P, 5], f32, tag="asb")
                    nc.vector.tensor_tensor(
                        out=a_sb[:], in0=a_ps[:, :5], in1=vb2_sb[:], op=add
                    )
                    velm = npool.tile([P, 15], f32, tag="velm")
                    nc.vector.tensor_tensor(
                        out=velm[:].rearrange("p (k v) -> p k v", v=3),
                        in0=velP_sb[:, nt * 15 : (nt + 1) * 15].rearrange(
                            "p (k v) -> p k v", v=3
                        ),
                        in1=a_sb[:].unsqueeze(-1).broadcast_to([P, 5, 3]),
                        op=mult,
                    )
                    nc.vector.tensor_reduce(
                        out=vc_acc[:, nt * 3 : (nt + 1) * 3],
                        in_=velm[:].rearrange("p (k v) -> p v k", v=3),
                        axis=mybir.AxisListType.X,
                        op=add,
                    )

            # ---- outputs ----
            nc.sync.dma_start(out=geomT[:, :], in_=geom_acc[:])
            nc.sync.dma_start(out=vc[:, :], in_=vc_acc[:])

    nc.compile()
    return nc


def _prep(h, m_ij, x, vel_all, edge_index, ew_W1, ew_b1, ew_W2, ew_b2,
          vg_W1, vg_b1, vg_W2, vg_b2):
    """Host-side sharding + layout packing. Returns (in_maps, meta)."""
    h = np.ascontiguousarray(np.asarray(h, dtype=np.float32))
    m_ij = np.ascontiguousarray(np.asarray(m_ij, dtype=np.float32))
    x = np.asarray(x, dtype=np.float32)
    vel_all = np.asarray(vel_all, dtype=np.float32)
    ei = np.asarray(edge_index)
    src = ei[0].astype(np.int64)
    dst = ei[1].astype(np.int64)

    N = h.shape[0]
    E = src.shape[0]

    W = int(np.ceil(N / (NC_CORES * P)))  # windows per core
    NK = W * P                            # nodes per core (padded)
    NPAD = NC_CORES * NK
    NKP = int(np.ceil(NK / 512)) * 512
    W_total = NC_CORES * W

    order = np.argsort(dst, kind="stable")
    dst_s = dst[order]
    cnt = np.bincount(dst, minlength=N).astype(np.float32)
    inv = 1.0 / np.maximum(cnt, 1.0)
    rel = (x[src] - x[dst]) * inv[dst][:, None]  # [E,3] with 1/cnt folded in

    wcnt = np.bincount(dst // P, minlength=W_total)
    T_w = max(int(np.ceil(wcnt.max() / P)), 1) if E > 0 else 1
    T = W * T_w
    EPAD = T * P

    win_starts = np.searchsorted(dst_s, np.arange(W_total) * P)
    offs = np.arange(T_w * P)
    slot_valid = offs[None, :] < wcnt[:, None]              # [W_total, T_w*P]
    slot_sorted = win_starts[:, None] + np.where(slot_valid, offs[None, :], 0)
    slot_sorted = np.minimum(slot_sorted, max(E - 1, 0))
    slot_eid = np.where(slot_valid, order[slot_sorted], -1)  # edge id or -1

    edt = np.float32 if F32_EDGE else ml_dtypes.bfloat16
    wt1 = np.ascontiguousarray(np.asarray(ew_W1, dtype=np.float32).astype(edt))
    wt2 = np.ascontiguousarray(
        np.asarray(ew_W2, dtype=np.float32).reshape(H, 1).astype(edt))
    vt1 = np.ascontiguousarray(np.asarray(vg_W1, dtype=np.float32).astype(edt))
    vt2 = np.ascontiguousarray(np.asarray(vg_W2, dtype=np.float32).reshape(H, 5).astype(edt))
    b1 = np.asarray(ew_b1, dtype=np.float32).reshape(H, 1)
    b2r = np.full((P, 1), np.float32(np.asarray(ew_b2).reshape(-1)[0]), np.float32)
    vb1 = np.asarray(vg_b1, dtype=np.float32).reshape(H, 1)
    vb2r = np.tile(np.asarray(vg_b2, dtype=np.float32).reshape(1, 5), (P, 1))

    h_pad = np.zeros((NPAD, H), np.float32)
    h_pad[:N] = h
    vel_pad = np.zeros((NPAD, 5, 3), np.float32)
    vel_pad[:N] = vel_all

    in_maps = []
    for k in range(NC_CORES):
        ids = slot_eid[k * W : (k + 1) * W].reshape(-1)  # [EPAD]
        valid = ids >= 0
        idc = np.where(valid, ids, 0)

        mg = m_ij[idc]
        mg[~valid] = 0.0
        mT = np.ascontiguousarray(mg.T.astype(edt))  # [H, EPAD]
        del mg

        rg = rel[idc]
        rg[~valid] = 0.0
        relP = np.ascontiguousarray(
            rg.reshape(T, P, 3).transpose(1, 0, 2).reshape(P, T * 3).astype(edt)
        )
        del rg

        base = (k * W + (np.arange(T) // T_w)) * P  # [T]
        dg = dst[idc].reshape(T, P) - base[:, None]
        dg[~valid.reshape(T, P)] = -1
        dstP = np.ascontiguousarray(dg.T.astype(np.float32).astype(edt))

        hT_k = np.zeros((H, NKP), edt)
        hT_k[:, :NK] = h_pad[k * NK : (k + 1) * NK].T.astype(edt)
        velP_k = np.ascontiguousarray(
            vel_pad[k * NK : (k + 1) * NK]
            .reshape(W, P, 15)
            .transpose(1, 0, 2)
            .reshape(P, W * 15)
        )

        in_maps.append({
            "mT": mT, "relP": relP, "dstP": dstP, "hT": hT_k, "velP": velP_k,
            "ew_W1": wt1, "ew_b1": b1, "ew_W2": wt2, "ew_b2r": b2r,
            "vg_W1": vt1, "vg_b1": vb1, "vg_W2": vt2, "vg_b2r": vb2r,
            "nonce": np.zeros((1, (int(_SELF_HASH, 16) % 509) + 2), np.float32),
        })

    meta = dict(N=N, W=W, T_w=T_w, NK=NK, NKP=NKP)
    return in_maps, meta


def kernel(**inputs):
    global LAST_EXEC_NS, LAST_RESULTS
    from concourse.bass_utils import run_bass_kernel_spmd

    in_maps, meta = _prep(**inputs)
    key = (meta["W"], meta["T_w"], meta["NKP"])
    if key not in _COMPILED:
        _COMPILED[key] = _build_program(*key)
    nc = _COMPILED[key]

    t0 = time.time()
    res = run_bass_kernel_spmd(
        nc, in_maps, core_ids=list(range(NC_CORES)), trace=TRACE
    )
    LAST_EXEC_NS = res.exec_time_ns
    LAST_RESULTS = res
    _ = time.time() - t0

    N, W, NK = meta["N"], meta["W"], meta["NK"]
    parts = []
    for k in range(NC_CORES):
        r = res.results[k]
        g = r["geomT"].reshape(3, NK).T  # [NK,3]
        v = r["vc"].reshape(P, W, 3).transpose(1, 0, 2).reshape(NK, 3)
        parts.append(g + v)
    out = np.concatenate(parts, axis=0)[:N]
    return out.astype(np.float32)


# revision 19
# speedup vs baseline: 1.2048x; 1.0057x over previous
"""EquivariantDecoder GNN message-passing kernel for 8 Trainium2 NeuronCores.

Strategy (destination-sharded, collective-free):
  - Host sorts edges by dst node and shards by dst-node ranges: core k owns
    nodes [k*NK, (k+1)*NK) and ALL edges pointing into them. Each core fully
    computes its output rows; no cross-core reduction is needed.
  - Per-edge MLP  w = silu(m @ W1 + b1) @ W2 + b2  runs with W1 stationary on
    the tensor engine over host-pre-transposed m (mT: [H, Epad]).
  - The scatter-mean becomes, per 128-node window, an accumulation of
    one-hot matmuls in PSUM:  geomT[v, n] += rel'[e, v] * (w[e] * 1[dst==n]),
    where rel' = (x[src]-x[dst]) / max(cnt[dst], 1) is host-prepared and the
    scaled one-hot is built on the vector engine in one fused
    tensor_scalar((IOTA == dstrel) * w) op per 128-edge tile.
  - Node-side velocity gating alpha = silu(h @ vgW1 + b1) @ vgW2 + b2,
    vel_combo = sum_k alpha[:,k] * vel_all[:,k,:] is node-parallel.
  - Host unpacks/adds the two per-core outputs and concatenates.
"""

import hashlib
import os
import sys
import time

import numpy as np

sys.path.insert(0, "/opt/trn_rl_repo")

import ml_dtypes

# Namespace the neuron compile cache by this file's content: the cache keys
# on HLO module hashes, which do not see BIR-level kernel changes.
_SELF_HASH = hashlib.sha256(open(__file__, "rb").read()).hexdigest()[:16]
os.environ.setdefault(
    "NEURON_COMPILE_CACHE_URL", f"/tmp/neuron-cache-{_SELF_HASH}"
)

NC_CORES = 8
P = 128
H = 128
F32_EDGE = bool(int(os.environ.get("KERNEL_F32", "0")))

_COMPILED = {}  # (W, T_w, NKP) -> nc
LAST_EXEC_NS = None
LAST_RESULTS = None
TRACE = bool(int(os.environ.get("KERNEL_TRACE", "0")))


def _build_program(W, T_w, NKP):
    """Build + compile the SPMD Tile program for one core.

    W    : 128-node windows per core
    T_w  : tiles (128 edges) per window (uniform, host-padded)
    NKP  : node columns per core padded to a multiple of 512
    """
    from concourse import bacc, mybir, tile

    T = W * T_w          # edge tiles per core
    EPAD = T * P         # padded edge count per core

    f32 = mybir.dt.float32
    i32 = mybir.dt.int32
    ebf = f32 if F32_EDGE else mybir.dt.bfloat16

    nc = bacc.Bacc(
        "TRN2", target_bir_lowering=False, debug=False, num_devices=NC_CORES
    )

    # ---- DRAM I/O ----
    mT = nc.dram_tensor("mT", [P, EPAD], ebf, kind="ExternalInput").ap()
    relP = nc.dram_tensor("relP", [P, T * 3], ebf, kind="ExternalInput").ap()
    dstP = nc.dram_tensor("dstP", [P, T], ebf, kind="ExternalInput").ap()
    hT = nc.dram_tensor("hT", [P, NKP], ebf, kind="ExternalInput").ap()
    velP = nc.dram_tensor("velP", [P, W * 15], f32, kind="ExternalInput").ap()
    ew_W1 = nc.dram_tensor("ew_W1", [P, H], ebf, kind="ExternalInput").ap()
    ew_b1 = nc.dram_tensor("ew_b1", [P, 1], f32, kind="ExternalInput").ap()
    ew_W2 = nc.dram_tensor("ew_W2", [P, 1], ebf, kind="ExternalInput").ap()
    ew_b2r = nc.dram_tensor("ew_b2r", [P, 1], f32, kind="ExternalInput").ap()
    vg_W1 = nc.dram_tensor("vg_W1", [P, H], ebf, kind="ExternalInput").ap()
    vg_b1 = nc.dram_tensor("vg_b1", [P, 1], f32, kind="ExternalInput").ap()
    vg_W2 = nc.dram_tensor("vg_W2", [P, 5], ebf, kind="ExternalInput").ap()
    vg_b2r = nc.dram_tensor("vg_b2r", [P, 5], f32, kind="ExternalInput").ap()
    geomT = nc.dram_tensor("geomT", [3, W * P], f32, kind="ExternalOutput").ap()
    vc = nc.dram_tensor("vc", [P, W * 3], f32, kind="ExternalOutput").ap()
    # Program-content nonce: makes the HLO fingerprint (and any HLO-keyed
    # executable cache) unique per kernel.py content.
    NONCE = (int(_SELF_HASH, 16) % 509) + 2
    nonce = nc.dram_tensor("nonce", [1, NONCE], f32, kind="ExternalInput").ap()

    MCH = 64       # mT chunk: 64 tiles = 8192 cols = 2 MiB bf16
    RCH = 128      # rel/dst chunk in tiles (must be a multiple of 4)

    Silu = mybir.ActivationFunctionType.Silu
    Copy = mybir.ActivationFunctionType.Copy
    add = mybir.AluOpType.add
    mult = mybir.AluOpType.mult
    is_equal = mybir.AluOpType.is_equal

    with tile.TileContext(nc) as tc:
        with (
            tc.tile_pool(name="const", bufs=1) as cpool,
            tc.tile_pool(name="mchunk", bufs=2) as mpool,
            tc.tile_pool(name="relchunk", bufs=2) as rpool,
            tc.tile_pool(name="silu", bufs=3) as spool,
            tc.tile_pool(name="wsb", bufs=3) as wpool,
            tc.tile_pool(name="oh", bufs=4) as ohpool,
            tc.tile_pool(name="acc", bufs=1) as accpool,
            tc.tile_pool(name="hchunk", bufs=2) as hpool,
            tc.tile_pool(name="nodesmall", bufs=3) as npool,
            tc.tile_pool(name="ps512", bufs=2, space="PSUM") as ps512,
            tc.tile_pool(name="pssmall", bufs=2, space="PSUM") as pssmall,
            tc.tile_pool(name="psgeom", bufs=2, space="PSUM") as psgeom,
        ):
            # ---- constants ----
            w1_sb = cpool.tile([P, H], ebf, tag="w1")
            nc.sync.dma_start(out=w1_sb[:], in_=ew_W1[:, :])
            b1_sb = cpool.tile([P, 1], f32, tag="b1")
            nc.sync.dma_start(out=b1_sb[:], in_=ew_b1[:, :])
            w2_sb = cpool.tile([P, 1], ebf, tag="w2")
            nc.sync.dma_start(out=w2_sb[:], in_=ew_W2[:, :])
            b2_sb = cpool.tile([P, 1], f32, tag="b2")
            nc.sync.dma_start(out=b2_sb[:], in_=ew_b2r[:, :])
            vw1_sb = cpool.tile([P, H], ebf, tag="vw1")
            nc.sync.dma_start(out=vw1_sb[:], in_=vg_W1[:, :])
            vb1_sb = cpool.tile([P, 1], f32, tag="vb1")
            nc.sync.dma_start(out=vb1_sb[:], in_=vg_b1[:, :])
            vw2_sb = cpool.tile([P, 5], ebf, tag="vw2")
            nc.sync.dma_start(out=vw2_sb[:], in_=vg_W2[:, :])
            vb2_sb = cpool.tile([P, 5], f32, tag="vb2")
            nc.sync.dma_start(out=vb2_sb[:], in_=vg_b2r[:, :])
            velP_sb = cpool.tile([P, W * 15], f32, tag="velp")
            nc.sync.dma_start(out=velP_sb[:], in_=velP[:, :])
            nonce_sb = cpool.tile([1, 512], f32, tag="nonce")
            nc.sync.dma_start(out=nonce_sb[:1, :NONCE], in_=nonce[:, :])

            # IOTA8: per-128 repeating iota over GT tiles, [128, GT*128]
            GT = 8  # tiles per edge group
            iota_i = cpool.tile([P, GT * P], i32, tag="iotai")
            nc.gpsimd.iota(
                iota_i[:], pattern=[[0, GT], [1, P]], base=0, channel_multiplier=0
            )
            iota_sb = cpool.tile([P, GT * P], ebf, tag="iotaf")
            nc.vector.tensor_copy(iota_sb[:], iota_i[:])

            geom_acc = accpool.tile([3, W * P], f32, tag="gacc")
            vc_acc = accpool.tile([P, W * 3], f32, tag="vacc")

            # ---- edge pipeline (groups of GT tiles = GT*128 edges) ----
            mch = None
            rch = None
            dch = None
            geom_ps = None
            for t0 in range(0, T, GT):
                gs = min(GT, T - t0)  # tiles in this group
                if t0 % MCH == 0:
                    mcols = min(MCH * P, EPAD - t0 * P)
                    mch = mpool.tile([P, MCH * P], ebf, tag="mch")
                    nc.sync.dma_start(
                        out=mch[:, :mcols], in_=mT[:, t0 * P : t0 * P + mcols]
                    )
                if t0 % RCH == 0:
                    rt = min(RCH, T - t0)
                    rch = rpool.tile([P, RCH * 3], ebf, tag="rch")
                    nc.sync.dma_start(
                        out=rch[:, : rt * 3], in_=relP[:, t0 * 3 : (t0 + rt) * 3]
                    )
                    dch = rpool.tile([P, RCH], ebf, tag="dch")
                    nc.sync.dma_start(out=dch[:, :rt], in_=dstP[:, t0 : t0 + rt])

                moff = (t0 % MCH) * P
                zT_ps = ps512.tile([P, GT * P], f32, tag="z512", space="PSUM")
                for c0 in range(0, gs * P, 512):  # one PSUM bank per matmul
                    cw = min(512, gs * P - c0)
                    nc.tensor.matmul(
                        out=zT_ps[:, c0 : c0 + cw],
                        lhsT=w1_sb[:],
                        rhs=mch[:, moff + c0 : moff + c0 + cw],
                        start=True,
                        stop=True,
                    )
                silu_sb = spool.tile([P, GT * P], ebf, tag="silu")
                nc.scalar.activation(
                    silu_sb[:, : gs * P], zT_ps[:, : gs * P], Silu, bias=b1_sb[:, :1]
                )

                w_ps = pssmall.tile([P, 8], f32, tag="wps", space="PSUM")
                for tt in range(gs):
                    nc.tensor.matmul(
                        out=w_ps[:, tt : tt + 1],
                        lhsT=silu_sb[:, tt * P : (tt + 1) * P],
                        rhs=w2_sb[:],
                        start=True,
                        stop=True,
                    )
                # relw[e, (t,c)] = (w_ps[e,t] + b2) * rel'[e, (t,c)]
                roff = (t0 % RCH) * 3
                relw_sb = wpool.tile([P, GT * 3], ebf, tag="relw")
                nc.vector.scalar_tensor_tensor(
                    out=relw_sb[:, : gs * 3].rearrange("p (t c) -> p t c", c=3),
                    in0=w_ps[:, :gs].unsqueeze(-1).broadcast_to([P, gs, 3]),
                    scalar=b2_sb[:, :1],
                    in1=rch[:, roff : roff + gs * 3].rearrange(
                        "p (t c) -> p t c", c=3
                    ),
                    op0=add,
                    op1=mult,
                )
                # eq[e, (t,n)] = (iota[n] == dstrel[e,t])  (one op per group)
                doff = t0 % RCH
                eq_sb = ohpool.tile([P, GT * P], ebf, tag="oh")
                nc.vector.tensor_tensor(
                    out=eq_sb[:, : gs * P].rearrange("p (t n) -> p t n", n=P),
                    in0=iota_sb[:, : gs * P].rearrange("p (t n) -> p t n", n=P),
                    in1=dch[:, doff : doff + gs]
                    .unsqueeze(-1)
                    .broadcast_to([P, gs, P]),
                    op=is_equal,
                )

                for tt in range(gs):
                    i = t0 + tt  # global tile
                    wwin = i // T_w
                    tin = i % T_w
                    if tin == 0:
                        geom_ps = psgeom.tile([3, P], f32, tag="gps", space="PSUM")
                    nc.tensor.matmul(
                        out=geom_ps[:],
                        lhsT=relw_sb[:, tt * 3 : (tt + 1) * 3],
                        rhs=eq_sb[:, tt * P : (tt + 1) * P],
                        start=(tin == 0),
                        stop=(tin == T_w - 1),
                    )
                    if tin == T_w - 1:
                        nc.scalar.activation(
                            geom_acc[:, wwin * P : (wwin + 1) * P],
                            geom_ps[:],
                            Copy,
                        )

            # ---- node pipeline ----
            NBN = NKP // 512
            HCH = 2048
            hch = None
            for b in range(NBN):
                c0 = b * 512
                if c0 % HCH == 0:
                    hcols = min(HCH, NKP - c0)
                    hch = hpool.tile([P, HCH], ebf, tag="hch")
                    nc.sync.dma_start(
                        out=hch[:, :hcols], in_=hT[:, c0 : c0 + hcols]
                    )
                hoff = c0 % HCH
                z2_ps = ps512.tile([P, 512], f32, tag="z512", space="PSUM")
                nc.tensor.matmul(
                    out=z2_ps[:],
                    lhsT=vw1_sb[:],
                    rhs=hch[:, hoff : hoff + 512],
                    start=True,
                    stop=True,
                )
                silu2_sb = spool.tile([P, 512], ebf, tag="silu2")
                nc.scalar.activation(silu2_sb[:], z2_ps[:], Silu, bias=vb1_sb[:, :1])
                for tt in range(4):
                    nt = b * 4 + tt  # node tile
                    if nt >= W:
                        break
                    a_ps = pssmall.tile([P, 8], f32, tag="wps", space="PSUM")
                    nc.tensor.matmul(
                        out=a_ps[:, :5],
                        lhsT=silu2_sb[:, tt * P : (tt + 1) * P],
                        rhs=vw2_sb[:],
                        start=True,
                        stop=True,
                    )
                    a_sb = npool.tile([P, 5], f32, tag="asb")
                    nc.vector.tensor_tensor(
                        out=a_sb[:], in0=a_ps[:, :5], in1=vb2_sb[:], op=add
                    )
                    velm = npool.tile([P, 15], f32, tag="velm")
                    nc.vector.tensor_tensor(
                        out=velm[:].rearrange("p (k v) -> p k v", v=3),
                        in0=velP_sb[:, nt * 15 : (nt + 1) * 15].rearrange(
                            "p (k v) -> p k v", v=3
                        ),
                        in1=a_sb[:].unsqueeze(-1).broadcast_to([P, 5, 3]),
                        op=mult,
                    )
                    nc.vector.tensor_reduce(
                        out=vc_acc[:, nt * 3 : (nt + 1) * 3],
                        in_=velm[:].rearrange("p (k v) -> p v k", v=3),
                        axis=mybir.AxisListType.X,
                        op=add,
                    )

            # ---- outputs ----
            nc.sync.dma_start(out=geomT[:, :], in_=geom_acc[:])
            nc.sync.dma_start(out=vc[:, :], in_=vc_acc[:])

    nc.compile()
    return nc


def _prep(h, m_ij, x, vel_all, edge_index, ew_W1, ew_b1, ew_W2, ew_b2,
          vg_W1, vg_b1, vg_W2, vg_b2):
    """Host-side sharding + layout packing. Returns (in_maps, meta)."""
    h = np.ascontiguousarray(np.asarray(h, dtype=np.float32))
    m_ij = np.ascontiguousarray(np.asarray(m_ij, dtype=np.float32))
    x = np.asarray(x, dtype=np.float32)
    vel_all = np.asarray(vel_all, dtype=np.float32)
    ei = np.asarray(edge_index)
    src = ei[0].astype(np.int64)
    dst = ei[1].astype(np.int64)

    N = h.shape[0]
    E = src.shape[0]

    W = int(np.ceil(N / (NC_CORES * P)))  # windows per core
    NK = W * P                            # nodes per core (padded)
    NPAD = NC_CORES * NK
    NKP = int(np.ceil(NK / 512)) * 512
    W_total = NC_CORES * W

    order = np.argsort(dst, kind="stable")
    dst_s = dst[order]
    cnt = np.bincount(dst, minlength=N).astype(np.float32)
    inv = 1.0 / np.maximum(cnt, 1.0)
    rel = (x[src] - x[dst]) * inv[dst][:, None]  # [E,3] with 1/cnt folded in

    wcnt = np.bincount(dst // P, minlength=W_total)
    T_w = max(int(np.ceil(wcnt.max() / P)), 1) if E > 0 else 1
    T = W * T_w
    EPAD = T * P

    win_starts = np.searchsorted(dst_s, np.arange(W_total) * P)
    offs = np.arange(T_w * P)
    slot_valid = offs[None, :] < wcnt[:, None]              # [W_total, T_w*P]
    slot_sorted = win_starts[:, None] + np.where(slot_valid, offs[None, :], 0)
    slot_sorted = np.minimum(slot_sorted, max(E - 1, 0))
    slot_eid = np.where(slot_valid, order[slot_sorted], -1)  # edge id or -1

    edt = np.float32 if F32_EDGE else ml_dtypes.bfloat16
    wt1 = np.ascontiguousarray(np.asarray(ew_W1, dtype=np.float32).astype(edt))
    wt2 = np.ascontiguousarray(
        np.asarray(ew_W2, dtype=np.float32).reshape(H, 1).astype(edt))
    vt1 = np.ascontiguousarray(np.asarray(vg_W1, dtype=np.float32).astype(edt))
    vt2 = np.ascontiguousarray(np.asarray(vg_W2, dtype=np.float32).reshape(H, 5).astype(edt))
    b1 = np.asarray(ew_b1, dtype=np.float32).reshape(H, 1)
    b2r = np.full((P, 1), np.float32(np.asarray(ew_b2).reshape(-1)[0]), np.float32)
    vb1 = np.asarray(vg_b1, dtype=np.float32).reshape(H, 1)
    vb2r = np.tile(np.asarray(vg_b2, dtype=np.float32).reshape(1, 5), (P, 1))

    h_pad = np.zeros((NPAD, H), np.float32)
    h_pad[:N] = h
    vel_pad = np.zeros((NPAD, 5, 3), np.float32)
    vel_pad[:N] = vel_all

    in_maps = []
    for k in range(NC_CORES):
        ids = slot_eid[k * W : (k + 1) * W].reshape(-1)  # [EPAD]
        valid = ids >= 0
        idc = np.where(valid, ids, 0)

        mg = m_ij[idc]
        mg[~valid] = 0.0
        mT = np.ascontiguousarray(mg.T.astype(edt))  # [H, EPAD]
        del mg

        rg = rel[idc]
        rg[~valid] = 0.0
        relP = np.ascontiguousarray(
            rg.reshape(T, P, 3).transpose(1, 0, 2).reshape(P, T * 3).astype(edt)
        )
        del rg

        base = (k * W + (np.arange(T) // T_w)) * P  # [T]
        dg = dst[idc].reshape(T, P) - base[:, None]
        dg[~valid.reshape(T, P)] = -1
        dstP = np.ascontiguousarray(dg.T.astype(np.float32).astype(edt))

        hT_k = np.zeros((H, NKP), edt)
        hT_k[:, :NK] = h_pad[k * NK : (k + 1) * NK].T.astype(edt)
        velP_k = np.ascontiguousarray(
            vel_pad[k * NK : (k + 1) * NK]
            .reshape(W, P, 15)
            .transpose(1, 0, 2)
            .reshape(P, W * 15)
        )

        in_maps.append({
            "mT": mT, "relP": relP, "dstP": dstP, "hT": hT_k, "velP": velP_k,
            "ew_W1": wt1, "ew_b1": b1, "ew_W2": wt2, "ew_b2r": b2r,
            "vg_W1": vt1, "vg_b1": vb1, "vg_W2": vt2, "vg_b2r": vb2r,
            "nonce": np.zeros((1, (int(_SELF_HASH, 16) % 509) + 2), np.float32),
        })

    meta = dict(N=N, W=W, T_w=T_w, NK=NK, NKP=NKP)
    return in_maps, meta


def kernel(**inputs):
    global LAST_EXEC_NS, LAST_RESULTS
    from concourse.bass_utils import run_bass_kernel_spmd

    in_maps, meta = _prep(**inputs)
    key = (meta["W"], meta["T_w"], meta["NKP"])
    if key not in _COMPILED:
        _COMPILED[key] = _build_program(*key)
    nc = _COMPILED[key]

    t0 = time.time()
    res = run_bass_kernel_spmd(
        nc, in_maps, core_ids=list(range(NC_CORES)), trace=TRACE
    )
    LAST_EXEC_NS = res.exec_time_ns
    LAST_RESULTS = res
    _ = time.time() - t0

    N, W, NK = meta["N"], meta["W"], meta["NK"]
    parts = []
    for k in range(NC_CORES):
        r = res.results[k]
        g = r["geomT"].reshape(3, NK).T  # [NK,3]
        v = r["vc"].reshape(P, W, 3).transpose(1, 0, 2).reshape(NK, 3)
        parts.append(g + v)
    out = np.concatenate(parts, axis=0)[:N]
    return out.astype(np.float32)
